# revision 31
# baseline (speedup 1.0000x reference)
"""Titans NeuralMemory forward on 8 Trainium2 NeuronCores.

Decomposition (validated vs reference):
  - Per-chunk MLP-loss gradients are rank-16: g_i = l_i^T r_i with l/r
    factors from a batched forward/backward pass at the base weights.
  - The two associative scans compose into a lower-triangular [64,64]
    T built stably in log space:
        T[t,j] = e^{cd_t} * (sum_k e^{-cd_k+mut} e^{cm_k+mls})[t,j] * e^{s_j}
    with the row/column exponential factors applied as per-partition
    scalar multiplies AFTER the inner matmul (no partition broadcasts).
  - Retrieval never materializes fast weights: per layer,
      X_{i+1} = silu(X_i @ W_i + (X_i @ L_i^T * M) @ R_i).

Layout: seq loaded contiguously (token t = 8p + i on partition p), so
store column l = i*128 + p has chunk(l) = p//2 independent of i: the
blockwise mask expansion collapses to a single [128,256] tile.

Sharding: 8 cores = 2 batch rows x 4 retrieve row-groups of 256 rows.
Matmuls in fp16 (FWL weight loads), accumulation in fp32 PSUM; the
scan/T pipeline stays fp32.
"""
import os
import numpy as np

import concourse.bass as bass
import concourse.tile as tile
from concourse import bacc, mybir
from concourse.bass_utils import run_bass_kernel_spmd

AF = mybir.ActivationFunctionType
ALU = mybir.AluOpType
FP32 = mybir.dt.float32
FP32R = mybir.dt.float32r
FP16 = mybir.dt.float16

B, L, D, C, DEPTH = 2, 1024, 256, 16, 4
N = L // C          # 64 chunks
P = 128
EPS = 1.1920929e-07
NCORES = 8
GROUPS = 4
RT = L // GROUPS    # 256 retrieve rows per core

# w32 blob layout (fp32 word offsets per partition)
WP_O = 0
UTN_O = WP_O + 8
NUTN_O = UTN_O + 128
MLS_O = NUTN_O + 128
MUT_O = MLS_O + 64
EL_O = MUT_O + 64
IEXP_O = EL_O + 128
W32_SZ = IEXP_O + 256

# w16 blob layout (fp16 element offsets per partition)
WQ_O = 0
WKV_O = WQ_O + 512
W_O = WKV_O + 1024
WT_O = W_O + 2048
W16_SZ = WT_O + 1536

_CACHE = {}
LAST_PERF = {}


def _install_ntff_hook():
    import sys
    import types
    try:
        from trn_agent_boot.trn_boot import _ntff_profile_via_ctypes
        hook = _ntff_profile_via_ctypes("/opt/axon/libaxon_pjrt.so")
    except Exception:
        return False
    if hook is None:
        return False
    mod = types.ModuleType("antenv.axon_hooks")
    mod.get_axon_ntff_profile_hook = lambda: hook
    mod.set_axon_ntff_profile_hook = lambda h: None
    sys.modules["antenv.axon_hooks"] = mod
    return True


def _build():
    nc = bacc.Bacc("TRN2", target_bir_lowering=False)

    seq_b = nc.dram_tensor("seq_b", [L, D], FP32, kind="ExternalInput")
    seq_q = nc.dram_tensor("seq_q", [RT, D], FP32, kind="ExternalInput")
    w32_d = nc.dram_tensor("w32_d", [P, W32_SZ], FP32R, kind="ExternalInput")
    w16_d = nc.dram_tensor("w16_d", [P, W16_SZ], FP16, kind="ExternalInput")
    w16s_d = nc.dram_tensor("w16s_d", [P, 128], FP16, kind="ExternalInput")
    out_d = nc.dram_tensor("out", [RT, D], FP32, kind="ExternalOutput")

    with tile.TileContext(nc) as tc:
        with (
            tc.tile_pool(name="big", bufs=1) as big,
            tc.tile_pool(name="rot", bufs=3) as rot,
            tc.tile_pool(name="pmm", bufs=3, space="PSUM") as pmm,
            tc.tile_pool(name="psc", bufs=3, space="PSUM") as psc,
            tc.tile_pool(name="ptr", bufs=2, space="PSUM") as ptr,
        ):
            # ---------------- bulk loads ----------------
            ident = big.tile([P, 128], FP16)
            nc.sync.dma_start(ident, w16s_d[:])
            sq8 = big.tile([P, 8, D], FP32, tag="sq8")
            src8 = seq_b[:].rearrange("(p i) d -> p i d", p=P)
            nc.sync.dma_start(sq8[:, 0:4, :], src8[:, 0:4, :])
            nc.sync.dma_start(sq8[:, 4:8, :], src8[:, 4:8, :])
            w32 = big.tile([P, W32_SZ], FP32R)
            nc.sync.dma_start(w32, w32_d[:])
            qs2 = big.tile([P, 2, D], FP32, tag="qs2")
            nc.sync.dma_start(qs2, seq_q[:].rearrange("(p i) d -> p i d", p=P))
            w16 = big.tile([P, W16_SZ], FP16)
            nc.sync.dma_start(w16, w16_d[:])

            wp_sb = w32[:, WP_O:WP_O + 8].rearrange("p (k m) -> p k m", k=2)
            utn_sb = w32[:, UTN_O:UTN_O + 128]
            nutn_sb = w32[:, NUTN_O:NUTN_O + 128]
            mls_sb = w32[:, MLS_O:MLS_O + 64]
            mut_sb = w32[:, MUT_O:MUT_O + 64]
            el_sb = w32[:, EL_O:EL_O + 128]
            iexp_sb = w32[:, IEXP_O:IEXP_O + 256]

            wq_sb = w16[:, WQ_O:WQ_O + 512].rearrange("p (k m) -> p k m", k=2)
            wkv_sb = w16[:, WKV_O:WKV_O + 1024].rearrange(
                "p (k m) -> p k m", k=2)
            w_sb = w16[:, W_O:W_O + 2048].rearrange(
                "p (l k m) -> p l k m", l=4, k=2)
            wt_sb = w16[:, WT_O:WT_O + 1536].rearrange(
                "p (l k m) -> p l k m", l=3, k=2)

            # ---------------- rmsnorms (scalar sq/sqrt + DVE aux) --------
            def rms_start(x, tag):
                scr = rot.tile([P, D], FP32, tag="rms_scr", bufs=2)
                ms = rot.tile([P, 1], FP32, tag=f"{tag}ms", bufs=2)
                nc.scalar.activation(scr, x, AF.Square, accum_out=ms)
                msd = rot.tile([P, 1], FP32, tag=f"{tag}md", bufs=2)
                nc.vector.tensor_scalar(
                    out=msd, in0=ms, scalar1=1.0 / D, scalar2=EPS,
                    op0=ALU.mult, op1=ALU.add)
                rv = rot.tile([P, 1], FP32, tag=f"{tag}rv", bufs=2)
                nc.vector.reciprocal(rv, msd)
                return rv

            def rms_fin(x, rv, tag, bufs=4):
                rstd = rot.tile([P, 1], FP32, tag=f"{tag}rs", bufs=2)
                nc.scalar.activation(rstd, rv, AF.Sqrt)
                out = rot.tile([P, D], FP16, tag=f"{tag}o", bufs=bufs)
                nc.vector.tensor_scalar_mul(out, x, rstd)
                return out

            sn = [None] * 8
            rq = [None] * 2
            rvs = {}
            for i in range(8):
                rvs[i] = rms_start(sq8[:, i, :], "sn")
                if i >= 2:
                    sn[i - 2] = rms_fin(sq8[:, i - 2, :], rvs[i - 2], "sn")
            for i in range(2):
                rvs[8 + i] = rms_start(qs2[:, i, :], "rq")
            sn[6] = rms_fin(sq8[:, 6, :], rvs[6], "sn")
            sn[7] = rms_fin(sq8[:, 7, :], rvs[7], "sn")
            for i in range(2):
                rq[i] = rms_fin(qs2[:, i, :], rvs[8 + i], "rq", bufs=2)

            # ---------------- transposes: snT ----------------
            snT = [big.tile([P, L], FP16, name=f"snT{k}") for k in range(2)]
            for grp in range(2):
                for ko in range(2):
                    tp = ptr.tile([P, 512], FP16, tag="tr")
                    for ii in range(4):
                        i = grp * 4 + ii
                        nc.tensor.transpose(
                            tp[:, ii * P:(ii + 1) * P],
                            sn[i][:, ko * P:(ko + 1) * P], ident)
                    nc.vector.tensor_copy(
                        snT[ko][:, grp * 512:(grp + 1) * 512], tp)

            # ---------------- chunk sums (2-step: tokens 8p+i) -----------
            # column l = i*128 + m, token = 8m + i, chunk = m//2
            cmT = big.tile([P, 2, N], FP32R)
            red8 = big.tile([P, 2, P], FP32)
            with nc.allow_low_precision(reason="fp32 accum of fp16 sums"):
                for ko in range(2):
                    nc.vector.reduce_sum(
                        red8[:, ko, :],
                        snT[ko].rearrange("p (i m) -> p m i", i=8),
                        axis=mybir.AxisListType.X)
                for ko in range(2):
                    nc.vector.tensor_add(
                        cmT[:, ko, :],
                        red8[:, ko, :].rearrange("p (n b) -> p n b", b=2)[:, :, 0],
                        red8[:, ko, :].rearrange("p (n b) -> p n b", b=2)[:, :, 1])

            # ---------------- zp (per-chunk gate logits) -----------------
            zp = ptr.tile([N, 4], FP32, tag="tr")
            for ko in range(2):
                nc.tensor.matmul(zp, cmT[:, ko, :], wp_sb[:, ko, :],
                                 start=(ko == 0), stop=(ko == 1))

            # kv projection (ko4: 0,1 -> kT; 2,3 -> vT)
            kT = [big.tile([P, L], FP16, name=f"kT{k}") for k in range(2)]
            vT = [big.tile([P, L], FP32, name=f"vT{k}") for k in range(2)]
            for ko4 in range(4):
                dest = kT[ko4] if ko4 < 2 else vT[ko4 - 2]
                for rc in range(2):
                    sl = slice(rc * 512, (rc + 1) * 512)
                    mm = pmm.tile([P, 512], FP32, tag="mm")
                    for ki in range(2):
                        nc.tensor.matmul(
                            mm, wkv_sb[:, ki, ko4 * P:(ko4 + 1) * P],
                            snT[ki][:, sl], start=(ki == 0), stop=(ki == 1))
                    if ko4 < 2:
                        nc.vector.tensor_copy(dest[:, sl], mm)
                    else:
                        nc.scalar.copy(dest[:, sl], mm)

            # sg: sigmoid gates (same table run as the fwd sigmoids below)
            # zp cols = [mom, decay, lr, pad]
            sg = big.tile([P, 4], FP32)
            nc.vector.memset(sg, 0.0)
            nc.scalar.activation(sg[:N, 0:4:2], zp[:, 0:4:2], AF.Sigmoid)
            nc.scalar.activation(sg[:N, 1:2], zp[:, 1:2], AF.Sigmoid,
                                 scale=-1.0)

            # ---------------- forward MLP ----------------
            Lf = [kT]
            dsT = []
            for i in range(3):
                a_next = [big.tile([P, L], FP16, name=f"aT{i+1}_{k}")
                          for k in range(2)]
                ds_i = [big.tile([P, L], FP16, name=f"dsT{i}_{k}")
                        for k in range(2)]
                for rc in range(2):
                    for mo in range(2):
                        sl = slice(rc * 512, (rc + 1) * 512)
                        mm = pmm.tile([P, 512], FP32, tag="mm")
                        for ki in range(2):
                            nc.tensor.matmul(
                                mm, w_sb[:, i, ki, mo * P:(mo + 1) * P],
                                Lf[i][ki][:, sl],
                                start=(ki == 0), stop=(ki == 1))
                        sgt = rot.tile([P, 512], FP16, tag="sgt", bufs=2)
                        nc.scalar.activation(sgt, mm, AF.Sigmoid)
                        nc.vector.tensor_mul(a_next[mo][:, sl], mm, sgt)
                        # t2 = 1 + h - a ; ds = sgt * t2 (gpsimd, off-path)
                        t2 = rot.tile([P, 512], FP16, tag="t2", bufs=2)
                        nc.vector.scalar_tensor_tensor(
                            out=t2, in0=mm, scalar=1.0,
                            in1=a_next[mo][:, sl],
                            op0=ALU.add, op1=ALU.subtract)
                        nc.gpsimd.tensor_mul(ds_i[mo][:, sl], sgt, t2)
                Lf.append(a_next)
                dsT.append(ds_i)

            # ---------------- pred + gg3 ----------------
            ggA = [big.tile([P, L], FP16, name=f"ggA{k}") for k in range(2)]
            ggB = [big.tile([P, L], FP16, name=f"ggB{k}") for k in range(2)]
            for rc in range(2):
                for mo in range(2):
                    sl = slice(rc * 512, (rc + 1) * 512)
                    mm = pmm.tile([P, 512], FP32, tag="mm")
                    for ki in range(2):
                        nc.tensor.matmul(
                            mm, w_sb[:, 3, ki, mo * P:(mo + 1) * P],
                            Lf[3][ki][:, sl], start=(ki == 0), stop=(ki == 1))
                    nc.vector.tensor_sub(ggA[mo][:, sl], vT[mo][:, sl], mm)

            # gate the ln/exp table switches past the last fwd sigmoid:
            # write sg[:,3] from a late fwd tile, ln reads sg[:,0:4]
            nc.vector.tensor_copy(sg[:N, 3:4], Lf[3][1][:N, 1023:1024])
            lg = big.tile([P, 4], FP32)
            nc.vector.memset(lg, 0.0)
            nc.scalar.activation(lg[:N, 0:4], sg[:N, 0:4], AF.Ln)
            lgr = big.tile([P, 2], FP32R)
            nc.vector.tensor_copy(lgr, lg[:, 0:2])

            # R factor emitter
            Rf = {i: [big.tile([P, D], FP16, name=f"Rf{i}_{jt}")
                      for jt in range(8)] for i in range(4)}

            def emit_R(layer, src):
                for jt in range(8):
                    tp = ptr.tile([P, 256], FP16, tag="tr")
                    for mo in range(2):
                        nc.tensor.transpose(
                            tp[:, mo * P:(mo + 1) * P],
                            src[mo][:, jt * P:(jt + 1) * P], ident)
                    if jt % 2 == 0:
                        nc.vector.tensor_copy(Rf[layer][jt], tp)
                    else:
                        nc.scalar.copy(Rf[layer][jt], tp)

            def bwd_layer(i, gg_cur, gg_next):
                for rc in range(2):
                    for mo in range(2):
                        sl = slice(rc * 512, (rc + 1) * 512)
                        mm = pmm.tile([P, 512], FP32, tag="mm")
                        for ki in range(2):
                            nc.tensor.matmul(
                                mm, wt_sb[:, i - 1, ki, mo * P:(mo + 1) * P],
                                gg_cur[ki][:, sl],
                                start=(ki == 0), stop=(ki == 1))
                        nc.vector.tensor_mul(
                            gg_next[mo][:, sl], mm, dsT[i - 1][mo][:, sl])

            emit_R(3, ggA)
            bwd_layer(3, ggA, ggB)

            # cacc/nacc (prefix sums of gate logs)
            cacc_p = ptr.tile([P, 2], FP32, tag="tr")
            nc.tensor.matmul(cacc_p, utn_sb, lgr, start=True, stop=True)
            cacc = big.tile([P, 2], FP32)
            nc.vector.tensor_copy(cacc, cacc_p)
            nacc_p = ptr.tile([P, 2], FP32, tag="tr")
            nc.tensor.matmul(nacc_p, nutn_sb, lgr, start=True, stop=True)
            nacc = big.tile([P, 2], FP32)
            nc.vector.tensor_copy(nacc, nacc_p)

            # exp run: T factors + rq rstd
            la = big.tile([P, N], FP32R)
            nc.scalar.activation(la, mls_sb, AF.Exp, bias=cacc[:, 0:1])
            ldt = big.tile([P, N], FP32R)
            nc.scalar.activation(ldt, mut_sb, AF.Exp, bias=nacc[:, 1:2])
            e1 = big.tile([P, 1], FP32)
            nc.scalar.activation(e1, cacc[:, 1:2], AF.Exp)
            eg = big.tile([P, 1], FP32)
            nc.scalar.activation(eg, lg[:, 2:3], AF.Exp, bias=nacc[:, 0:1])

            emit_R(2, ggB)
            bwd_layer(2, ggB, ggA)

            # T matrix: tt -> ttile -> texp -> mask
            tt_p = ptr.tile([N, N], FP32, tag="tr")
            nc.tensor.matmul(tt_p, ldt, la, start=True, stop=True)
            ttile = big.tile([N, N], FP32R)
            nc.vector.tensor_scalar_mul(ttile, tt_p, e1[:N])
            texp_p = ptr.tile([N, 256], FP32, tag="tr")
            nc.tensor.matmul(texp_p, ttile, iexp_sb[:N], start=True, stop=True)
            texp = big.tile([N, 256], FP32R)
            nc.vector.tensor_scalar_mul(texp, texp_p, eg[:N])

            emit_R(1, ggA)
            bwd_layer(1, ggA, ggB)

            mb_p = ptr.tile([P, 256], FP32, tag="tr")
            nc.tensor.matmul(mb_p, el_sb[:N], texp, start=True, stop=True)
            maskbx = big.tile([P, 512], FP32)
            nc.vector.tensor_copy(maskbx[:, 0:256], mb_p)
            nc.vector.tensor_copy(maskbx[:, 256:512], mb_p)

            emit_R(0, ggB)

            # ---------------- rqT + q projection ----------------
            rqT = [big.tile([P, RT], FP16, name=f"rqT{k}") for k in range(2)]
            for ko in range(2):
                tp = ptr.tile([P, 256], FP16, tag="tr")
                for rt in range(2):
                    nc.tensor.transpose(
                        tp[:, rt * P:(rt + 1) * P],
                        rq[rt][:, ko * P:(ko + 1) * P], ident)
                nc.vector.tensor_copy(rqT[ko], tp)
            XTa = big.tile([P, 2, RT], FP16, name="XTa")
            qp = psc.tile([P, 512], FP32, tag="sc")
            for ki in range(2):
                for kin in range(2):
                    nc.tensor.matmul(
                        qp[:, ki * 256:(ki + 1) * 256],
                        wq_sb[:, kin, ki * P:(ki + 1) * P], rqT[kin],
                        start=(kin == 0), stop=(kin == 1))
            nc.vector.tensor_copy(XTa.rearrange("p k r -> p (k r)"), qp)

            # ---------------- retrieve ----------------
            XTb = big.tile([P, 2, RT], FP16, name="XTb")
            X4T = big.tile([P, 2, RT], FP16, name="X4T")
            XTin, XTout = XTa, XTb
            last_tgt = None
            for i in range(4):
                msc = []
                for pr in range(4):
                    sc = psc.tile([P, 512], FP32, tag="sc")
                    for jj in range(2):
                        jt = pr * 2 + jj
                        for ki in range(2):
                            nc.tensor.matmul(
                                sc[:, jj * 256:(jj + 1) * 256],
                                Lf[i][ki][:, jt * P:(jt + 1) * P],
                                XTin[:, ki, :],
                                start=(ki == 0), stop=(ki == 1))
                    m = rot.tile([P, 512], FP16, tag="msc", bufs=4)
                    nc.vector.tensor_mul(m, sc, maskbx)
                    msc.append(m)
                y = pmm.tile([P, 2, RT], FP32, tag="mm")
                for mo in range(2):
                    for ki in range(2):
                        nc.tensor.matmul(
                            y[:, mo, :], w_sb[:, i, ki, mo * P:(mo + 1) * P],
                            XTin[:, ki, :],
                            start=(ki == 0), stop=False)
                    for jt in range(8):
                        nc.tensor.matmul(
                            y[:, mo, :], Rf[i][jt][:, mo * P:(mo + 1) * P],
                            msc[jt // 2][:, (jt % 2) * 256:(jt % 2 + 1) * 256],
                            start=False, stop=(jt == 7))
                if i < 3:
                    # silu via tanh (same act table as exp): X_stored picks up
                    # a 2x per layer, compensated in the next tanh's scale and
                    # cancelled by the scale-invariant postnorm at the end
                    for mo in range(2):
                        tgt = rot.tile([P, RT], FP16, tag="sgr", bufs=2)
                        nc.scalar.activation(tgt, y[:, mo, :], AF.Tanh,
                                             scale=0.5 / (2.0 ** i))
                        last_tgt = tgt
                        nc.vector.scalar_tensor_tensor(
                            out=XTout[:, mo, :], in0=tgt, scalar=1.0,
                            in1=y[:, mo, :], op0=ALU.add, op1=ALU.mult)
                else:
                    nc.vector.tensor_copy(
                        X4T.rearrange("p k r -> p (k r)"), y.rearrange(
                            "p k r -> p (k r)"))
                XTin, XTout = XTout, XTin

            # ---------------- postnorm + output ----------------
            # dummy sqrt depending on the last tanh: pulls the sqrt-set
            # table load off the tail chain but no earlier than retrieve L2
            dmy = rot.tile([P, 1], FP32, tag="dmy", bufs=1)
            nc.scalar.activation(dmy, last_tgt[:, 0:1], AF.Sqrt)
            out_rr = out_d[:].rearrange("(p i) d -> p i d", p=P)
            for rt in range(2):
                tp = ptr.tile([P, 256], FP16, tag="tr")
                for ko in range(2):
                    nc.tensor.transpose(
                        tp[:, ko * P:(ko + 1) * P],
                        X4T[:, ko, rt * P:(rt + 1) * P], ident)
                x4 = rot.tile([P, D], FP32, tag="x4", bufs=2)
                nc.vector.tensor_copy(x4, tp)
                rv = rms_start(x4, "pn")
                rstd = rot.tile([P, 1], FP32, tag="pnrs", bufs=2)
                nc.scalar.activation(rstd, rv, AF.Sqrt)
                o = rot.tile([P, D], FP32, tag="osb", bufs=2)
                nc.vector.tensor_scalar_mul(o, x4, rstd)
                nc.sync.dma_start(out_rr[:, rt, :], o)

    nc.compile()
    return nc


def _host_prep(inputs):
    seq = np.ascontiguousarray(np.asarray(inputs["seq"], dtype=np.float32))
    Wq = np.asarray(inputs["Wq"], dtype=np.float32)
    Wkv = np.asarray(inputs["Wkv"], dtype=np.float32)
    Ws = [np.asarray(inputs[f"W{i}"], dtype=np.float32) for i in range(4)]
    wa = np.asarray(inputs["w_adapt"], dtype=np.float32)
    wm = np.asarray(inputs["w_mom"], dtype=np.float32)
    wd = np.asarray(inputs["w_decay"], dtype=np.float32)

    def kxm(w):  # [K, M] -> [128, (K/128)*M]
        return w.reshape(w.shape[0] // P, P, w.shape[1]).transpose(1, 0, 2) \
            .reshape(P, -1)

    ii = np.arange(N)
    low = (ii[:, None] <= ii[None, :]).astype(np.float32)  # k <= t

    # zp cols: [mom, decay, lr, pad]
    wpack = np.zeros((D, 4), np.float32)
    wpack[:, 0] = wm
    wpack[:, 1] = wd
    wpack[:, 2] = wa
    wpack *= (1.0 / C)

    w32 = np.zeros((P, W32_SZ), np.float32)
    w32[:, WP_O:WP_O + 8] = kxm(wpack)
    w32[:N, UTN_O:UTN_O + N] = low
    w32[:N, NUTN_O:NUTN_O + N] = -low
    mls = np.full((P, N), -1e30, np.float32)
    mls[:N] = np.where(ii[:, None] >= ii[None, :], 0.0, -1e30)
    w32[:, MLS_O:MLS_O + N] = mls
    mut = np.full((P, N), -1e30, np.float32)
    mut[:N] = np.where(ii[:, None] <= ii[None, :], 0.0, -1e30)
    w32[:, MUT_O:MUT_O + N] = mut
    pp = np.arange(P)
    w32[:N, EL_O:EL_O + P] = (ii[:, None] == (pp[None, :] // 2)) \
        .astype(np.float32)

    w16 = np.zeros((P, W16_SZ), np.float16)
    w16[:, WQ_O:WQ_O + 512] = kxm(Wq).astype(np.float16)
    w16[:, WKV_O:WKV_O + 1024] = kxm(Wkv).astype(np.float16)
    w_all = np.stack(Ws).reshape(4, 2, P, D).transpose(2, 0, 1, 3)
    w16[:, W_O:W_O + 2048] = w_all.reshape(P, -1).astype(np.float16)
    wt_all = np.stack([Ws[1].T, Ws[2].T, Ws[3].T]) \
        .reshape(3, 2, P, D).transpose(2, 0, 1, 3)
    w16[:, WT_O:WT_O + 1536] = wt_all.reshape(P, -1).astype(np.float16)
    w16s = np.eye(P, dtype=np.float16)

    rr = np.arange(RT)
    tok = 2 * (rr % P) + rr // P  # retrieve column r -> local query token

    in_maps = []
    for core in range(NCORES):
        b, g = divmod(core, GROUPS)
        j0 = RT * g + (C - 1)
        w32_c = w32.copy()
        gchunk = (RT * g + tok) // C
        iexp = np.zeros((P, RT), np.float32)
        valid = (j0 + tok) < L
        iexp[gchunk[valid], rr[valid]] = 2.0 / D
        w32_c[:, IEXP_O:IEXP_O + RT] = iexp
        qs = np.zeros((RT, D), np.float32)
        src = seq[b, j0:min(j0 + RT, L)]
        qs[:len(src)] = src
        in_maps.append({"w32_d": w32_c, "w16_d": w16, "w16s_d": w16s,
                        "seq_b": seq[b], "seq_q": qs})
    return in_maps


def kernel(**inputs):
    if "nc" not in _CACHE:
        _CACHE["nc"] = _build()
    nc = _CACHE["nc"]
    in_maps = _host_prep(inputs)
    trace = bool(int(os.environ.get("KERNEL_TRACE", "0")))
    if trace:
        try:
            from antenv.axon_hooks import get_axon_ntff_profile_hook  # noqa: F401
        except ImportError:
            trace = _install_ntff_hook()
    res = run_bass_kernel_spmd(
        nc, in_maps, core_ids=list(range(NCORES)), trace=trace)
    LAST_PERF.clear()
    LAST_PERF.update(dict(
        exec_time_ns=res.exec_time_ns,
        mean_exec_time_ns=res.mean_exec_time_ns,
        profile_json=res.profile_json,
        trace=res.instructions_and_trace[1] if res.instructions_and_trace else None,
    ))
    final = np.zeros((B, L, D), np.float32)
    rr = np.arange(RT)
    tok = 2 * (rr % P) + rr // P
    for core in range(NCORES):
        b, g = divmod(core, GROUPS)
        j0 = RT * g + (C - 1)
        n = min(RT, L - j0)
        # out rows are already in token order via the permuted store AP
        final[b, j0:j0 + n] = res.results[core]["out"][:n]
    return final


# revision 33
# speedup vs baseline: 1.2487x; 1.2487x over previous
"""Titans NeuralMemory forward on 8 Trainium2 NeuronCores.

Decomposition (validated vs reference):
  - Per-chunk MLP-loss gradients are rank-16: g_i = l_i^T r_i with l/r
    factors from a batched forward/backward pass at the base weights.
  - The two associative scans compose into a lower-triangular [64,64]
    T built stably in log space:
        T[t,j] = e^{cd_t} * (sum_k e^{-cd_k+mut} e^{cm_k+mls})[t,j] * e^{s_j}
    with the row/column exponential factors applied as per-partition
    scalar multiplies AFTER the inner matmul (no partition broadcasts).
  - Retrieval never materializes fast weights: per layer,
      X_{i+1} = silu(X_i @ W_i + (X_i @ L_i^T * M) @ R_i).

Layout: seq loaded contiguously (token t = 8p + i on partition p), so
store column l = i*128 + p has chunk(l) = p//2 independent of i: the
blockwise mask expansion collapses to a single [128,256] tile.

Sharding: 8 cores = 2 batch rows x 4 retrieve row-groups of 256 rows.
Matmuls in fp16 (FWL weight loads), accumulation in fp32 PSUM; the
scan/T pipeline stays fp32.
"""
import os
import numpy as np

import concourse.bass as bass
import concourse.tile as tile
from concourse import bacc, mybir
from concourse.bass_utils import run_bass_kernel_spmd

AF = mybir.ActivationFunctionType
ALU = mybir.AluOpType
FP32 = mybir.dt.float32
FP32R = mybir.dt.float32r
FP16 = mybir.dt.float16

B, L, D, C, DEPTH = 2, 1024, 256, 16, 4
N = L // C          # 64 chunks
P = 128
EPS = 1.1920929e-07
NCORES = 8
GROUPS = 4
RT = L // GROUPS    # 256 retrieve rows per core

# w32 blob layout (fp32 word offsets per partition)
WP_O = 0
UTN_O = WP_O + 8
NUTN_O = UTN_O + 128
MLS_O = NUTN_O + 128
MUT_O = MLS_O + 64
EL_O = MUT_O + 64
IEXP_O = EL_O + 128
W32_SZ = IEXP_O + 256

# w16 blob layout (fp16 element offsets per partition)
WQ_O = 0
WKV_O = WQ_O + 512
W_O = WKV_O + 1024
WT_O = W_O + 2048
W16_SZ = WT_O + 1536

_CACHE = {}
LAST_PERF = {}


def _install_ntff_hook():
    import sys
    import types
    try:
        from trn_agent_boot.trn_boot import _ntff_profile_via_ctypes
        hook = _ntff_profile_via_ctypes("/opt/axon/libaxon_pjrt.so")
    except Exception:
        return False
    if hook is None:
        return False
    mod = types.ModuleType("antenv.axon_hooks")
    mod.get_axon_ntff_profile_hook = lambda: hook
    mod.set_axon_ntff_profile_hook = lambda h: None
    sys.modules["antenv.axon_hooks"] = mod
    return True


def _build():
    nc = bacc.Bacc("TRN2", target_bir_lowering=False)

    seq_b = nc.dram_tensor("seq_b", [L, D], FP32, kind="ExternalInput")
    seq_q = nc.dram_tensor("seq_q", [RT, D], FP32, kind="ExternalInput")
    w32_d = nc.dram_tensor("w32_d", [P, W32_SZ], FP32R, kind="ExternalInput")
    w16_d = nc.dram_tensor("w16_d", [P, W16_SZ], FP16, kind="ExternalInput")
    w16s_d = nc.dram_tensor("w16s_d", [P, 128], FP16, kind="ExternalInput")
    out_d = nc.dram_tensor("out", [RT, D], FP32, kind="ExternalOutput")

    with tile.TileContext(nc) as tc:
        with (
            tc.tile_pool(name="big", bufs=1) as big,
            tc.tile_pool(name="rot", bufs=3) as rot,
            tc.tile_pool(name="pmm", bufs=3, space="PSUM") as pmm,
            tc.tile_pool(name="psc", bufs=3, space="PSUM") as psc,
            tc.tile_pool(name="ptr", bufs=2, space="PSUM") as ptr,
        ):
            # ---------------- bulk loads ----------------
            ident = big.tile([P, 128], FP16)
            nc.sync.dma_start(ident, w16s_d[:])
            sq8 = big.tile([P, 8, D], FP32, tag="sq8")
            src8 = seq_b[:].rearrange("(p i) d -> p i d", p=P)
            nc.sync.dma_start(sq8[:, 0:4, :], src8[:, 0:4, :])
            nc.sync.dma_start(sq8[:, 4:8, :], src8[:, 4:8, :])
            w32 = big.tile([P, W32_SZ], FP32R)
            nc.sync.dma_start(w32, w32_d[:])
            qs2 = big.tile([P, 2, D], FP32, tag="qs2")
            nc.sync.dma_start(qs2, seq_q[:].rearrange("(p i) d -> p i d", p=P))
            w16 = big.tile([P, W16_SZ], FP16)
            nc.sync.dma_start(w16, w16_d[:])

            wp_sb = w32[:, WP_O:WP_O + 8].rearrange("p (k m) -> p k m", k=2)
            utn_sb = w32[:, UTN_O:UTN_O + 128]
            nutn_sb = w32[:, NUTN_O:NUTN_O + 128]
            mls_sb = w32[:, MLS_O:MLS_O + 64]
            mut_sb = w32[:, MUT_O:MUT_O + 64]
            el_sb = w32[:, EL_O:EL_O + 128]
            iexp_sb = w32[:, IEXP_O:IEXP_O + 256]

            wq_sb = w16[:, WQ_O:WQ_O + 512].rearrange("p (k m) -> p k m", k=2)
            wkv_sb = w16[:, WKV_O:WKV_O + 1024].rearrange(
                "p (k m) -> p k m", k=2)
            w_sb = w16[:, W_O:W_O + 2048].rearrange(
                "p (l k m) -> p l k m", l=4, k=2)
            wt_sb = w16[:, WT_O:WT_O + 1536].rearrange(
                "p (l k m) -> p l k m", l=3, k=2)

            # ---------------- rmsnorms (scalar sq/sqrt + DVE aux) --------
            def rms_start(x, tag):
                scr = rot.tile([P, D], FP32, tag="rms_scr", bufs=2)
                ms = rot.tile([P, 1], FP32, tag=f"{tag}ms", bufs=2)
                nc.scalar.activation(scr, x, AF.Square, accum_out=ms)
                msd = rot.tile([P, 1], FP32, tag=f"{tag}md", bufs=2)
                nc.vector.tensor_scalar(
                    out=msd, in0=ms, scalar1=1.0 / D, scalar2=EPS,
                    op0=ALU.mult, op1=ALU.add)
                rv = rot.tile([P, 1], FP32, tag=f"{tag}rv", bufs=2)
                nc.vector.reciprocal(rv, msd)
                return rv

            def rms_fin(x, rv, tag, bufs=4):
                rstd = rot.tile([P, 1], FP32, tag=f"{tag}rs", bufs=2)
                nc.scalar.activation(rstd, rv, AF.Sqrt)
                out = rot.tile([P, D], FP16, tag=f"{tag}o", bufs=bufs)
                nc.vector.tensor_scalar_mul(out, x, rstd)
                return out

            sn = [None] * 8
            rq = [None] * 2
            rvs = {}
            for i in range(8):
                rvs[i] = rms_start(sq8[:, i, :], "sn")
                if i >= 2:
                    sn[i - 2] = rms_fin(sq8[:, i - 2, :], rvs[i - 2], "sn")
            for i in range(2):
                rvs[8 + i] = rms_start(qs2[:, i, :], "rq")
            sn[6] = rms_fin(sq8[:, 6, :], rvs[6], "sn")
            sn[7] = rms_fin(sq8[:, 7, :], rvs[7], "sn")
            for i in range(2):
                rq[i] = rms_fin(qs2[:, i, :], rvs[8 + i], "rq", bufs=2)

            # ---------------- transposes: snT ----------------
            snT = [big.tile([P, L], FP16, name=f"snT{k}") for k in range(2)]
            for grp in range(2):
                for ko in range(2):
                    tp = ptr.tile([P, 512], FP16, tag="tr")
                    for ii in range(4):
                        i = grp * 4 + ii
                        nc.tensor.transpose(
                            tp[:, ii * P:(ii + 1) * P],
                            sn[i][:, ko * P:(ko + 1) * P], ident)
                    nc.vector.tensor_copy(
                        snT[ko][:, grp * 512:(grp + 1) * 512], tp)

            # ---------------- chunk sums (2-step: tokens 8p+i) -----------
            # column l = i*128 + m, token = 8m + i, chunk = m//2
            cmT = big.tile([P, 2, N], FP32R)
            red8 = big.tile([P, 2, P], FP32)
            with nc.allow_low_precision(reason="fp32 accum of fp16 sums"):
                for ko in range(2):
                    nc.vector.reduce_sum(
                        red8[:, ko, :],
                        snT[ko].rearrange("p (i m) -> p m i", i=8),
                        axis=mybir.AxisListType.X)
                for ko in range(2):
                    nc.vector.tensor_add(
                        cmT[:, ko, :],
                        red8[:, ko, :].rearrange("p (n b) -> p n b", b=2)[:, :, 0],
                        red8[:, ko, :].rearrange("p (n b) -> p n b", b=2)[:, :, 1])

            # ---------------- zp (per-chunk gate logits) -----------------
            zp = ptr.tile([N, 4], FP32, tag="tr")
            for ko in range(2):
                nc.tensor.matmul(zp, cmT[:, ko, :], wp_sb[:, ko, :],
                                 start=(ko == 0), stop=(ko == 1))

            # kv projection (ko4: 0,1 -> kT; 2,3 -> vT)
            kT = [big.tile([P, L], FP16, name=f"kT{k}") for k in range(2)]
            vT = [big.tile([P, L], FP32, name=f"vT{k}") for k in range(2)]
            for ko4 in range(4):
                dest = kT[ko4] if ko4 < 2 else vT[ko4 - 2]
                for rc in range(2):
                    sl = slice(rc * 512, (rc + 1) * 512)
                    mm = pmm.tile([P, 512], FP32, tag="mm")
                    for ki in range(2):
                        nc.tensor.matmul(
                            mm, wkv_sb[:, ki, ko4 * P:(ko4 + 1) * P],
                            snT[ki][:, sl], start=(ki == 0), stop=(ki == 1))
                    if ko4 < 2:
                        nc.vector.tensor_copy(dest[:, sl], mm)
                    else:
                        nc.scalar.copy(dest[:, sl], mm)

            # sg: sigmoid gates (same table run as the fwd sigmoids below)
            # zp cols = [mom, decay, lr, pad]
            sg = big.tile([P, 4], FP32)
            nc.vector.memset(sg, 0.0)
            nc.scalar.activation(sg[:N, 0:4:2], zp[:, 0:4:2], AF.Sigmoid)
            nc.scalar.activation(sg[:N, 1:2], zp[:, 1:2], AF.Sigmoid,
                                 scale=-1.0)

            # ---------------- forward MLP ----------------
            Lf = [kT]
            dsT = []
            for i in range(3):
                a_next = [big.tile([P, L], FP16, name=f"aT{i+1}_{k}")
                          for k in range(2)]
                ds_i = [big.tile([P, L], FP16, name=f"dsT{i}_{k}")
                        for k in range(2)]
                for rc in range(2):
                    for mo in range(2):
                        sl = slice(rc * 512, (rc + 1) * 512)
                        mm = pmm.tile([P, 512], FP32, tag="mm")
                        for ki in range(2):
                            nc.tensor.matmul(
                                mm, w_sb[:, i, ki, mo * P:(mo + 1) * P],
                                Lf[i][ki][:, sl],
                                start=(ki == 0), stop=(ki == 1))
                        sgt = rot.tile([P, 512], FP16, tag="sgt", bufs=4)
                        nc.scalar.activation(sgt, mm, AF.Sigmoid)
                        nc.vector.tensor_mul(a_next[mo][:, sl], mm, sgt)
                        # t2 = 1 + h - a ; ds = sgt * t2 (split vec/gpsimd)
                        t2 = rot.tile([P, 512], FP16, tag="t2", bufs=4)
                        nc.vector.scalar_tensor_tensor(
                            out=t2, in0=mm, scalar=1.0,
                            in1=a_next[mo][:, sl],
                            op0=ALU.add, op1=ALU.subtract)
                        if mo == 0:
                            nc.gpsimd.tensor_mul(ds_i[mo][:, sl], sgt, t2)
                        else:
                            nc.vector.tensor_mul(ds_i[mo][:, sl], sgt, t2)
                Lf.append(a_next)
                dsT.append(ds_i)

            # ---------------- pred + gg3 ----------------
            ggA = [big.tile([P, L], FP16, name=f"ggA{k}") for k in range(2)]
            ggB = [big.tile([P, L], FP16, name=f"ggB{k}") for k in range(2)]
            for rc in range(2):
                for mo in range(2):
                    sl = slice(rc * 512, (rc + 1) * 512)
                    mm = pmm.tile([P, 512], FP32, tag="mm")
                    for ki in range(2):
                        nc.tensor.matmul(
                            mm, w_sb[:, 3, ki, mo * P:(mo + 1) * P],
                            Lf[3][ki][:, sl], start=(ki == 0), stop=(ki == 1))
                    nc.vector.tensor_sub(ggA[mo][:, sl], vT[mo][:, sl], mm)

            lg = big.tile([P, 4], FP32)
            nc.vector.memset(lg, 0.0)
            nc.scalar.activation(lg[:N, 0:3], sg[:N, 0:3], AF.Ln)
            lgr = big.tile([P, 2], FP32R)
            nc.vector.tensor_copy(lgr, lg[:, 0:2])

            # R factor emitter
            Rf = {i: [big.tile([P, D], FP16, name=f"Rf{i}_{jt}")
                      for jt in range(8)] for i in range(4)}

            def emit_R(layer, src):
                for jt in range(8):
                    tp = ptr.tile([P, 256], FP16, tag="tr")
                    for mo in range(2):
                        nc.tensor.transpose(
                            tp[:, mo * P:(mo + 1) * P],
                            src[mo][:, jt * P:(jt + 1) * P], ident)
                    if jt % 2 == 0:
                        nc.vector.tensor_copy(Rf[layer][jt], tp)
                    else:
                        nc.scalar.copy(Rf[layer][jt], tp)

            def bwd_layer(i, gg_cur, gg_next):
                for rc in range(2):
                    for mo in range(2):
                        sl = slice(rc * 512, (rc + 1) * 512)
                        mm = pmm.tile([P, 512], FP32, tag="mm")
                        for ki in range(2):
                            nc.tensor.matmul(
                                mm, wt_sb[:, i - 1, ki, mo * P:(mo + 1) * P],
                                gg_cur[ki][:, sl],
                                start=(ki == 0), stop=(ki == 1))
                        nc.vector.tensor_mul(
                            gg_next[mo][:, sl], mm, dsT[i - 1][mo][:, sl])

            emit_R(3, ggA)
            bwd_layer(3, ggA, ggB)

            # cacc/nacc (prefix sums of gate logs)
            cacc_p = ptr.tile([P, 2], FP32, tag="tr")
            nc.tensor.matmul(cacc_p, utn_sb, lgr, start=True, stop=True)
            cacc = big.tile([P, 2], FP32)
            nc.vector.tensor_copy(cacc, cacc_p)
            nacc_p = ptr.tile([P, 2], FP32, tag="tr")
            nc.tensor.matmul(nacc_p, nutn_sb, lgr, start=True, stop=True)
            nacc = big.tile([P, 2], FP32)
            nc.vector.tensor_copy(nacc, nacc_p)

            # exp run: T factors + rq rstd
            la = big.tile([P, N], FP32R)
            nc.scalar.activation(la, mls_sb, AF.Exp, bias=cacc[:, 0:1])
            ldt = big.tile([P, N], FP32R)
            nc.scalar.activation(ldt, mut_sb, AF.Exp, bias=nacc[:, 1:2])
            e1 = big.tile([P, 1], FP32)
            nc.scalar.activation(e1, cacc[:, 1:2], AF.Exp)
            eg = big.tile([P, 1], FP32)
            nc.scalar.activation(eg, lg[:, 2:3], AF.Exp, bias=nacc[:, 0:1])

            emit_R(2, ggB)
            bwd_layer(2, ggB, ggA)

            # T matrix: tt -> ttile -> texp -> mask
            tt_p = ptr.tile([N, N], FP32, tag="tr")
            nc.tensor.matmul(tt_p, ldt, la, start=True, stop=True)
            ttile = big.tile([N, N], FP32R)
            nc.vector.tensor_scalar_mul(ttile, tt_p, e1[:N])
            texp_p = ptr.tile([N, 256], FP32, tag="tr")
            nc.tensor.matmul(texp_p, ttile, iexp_sb[:N], start=True, stop=True)
            texp = big.tile([N, 256], FP32R)
            nc.vector.tensor_scalar_mul(texp, texp_p, eg[:N])

            emit_R(1, ggA)
            bwd_layer(1, ggA, ggB)

            mb_p = ptr.tile([P, 256], FP32, tag="tr")
            nc.tensor.matmul(mb_p, el_sb[:N], texp, start=True, stop=True)
            maskbx = big.tile([P, 512], FP32)
            nc.vector.tensor_copy(maskbx[:, 0:256], mb_p)
            nc.vector.tensor_copy(maskbx[:, 256:512], mb_p)

            emit_R(0, ggB)

            # ---------------- rqT + q projection ----------------
            rqT = [big.tile([P, RT], FP16, name=f"rqT{k}") for k in range(2)]
            for ko in range(2):
                tp = ptr.tile([P, 256], FP16, tag="tr")
                for rt in range(2):
                    nc.tensor.transpose(
                        tp[:, rt * P:(rt + 1) * P],
                        rq[rt][:, ko * P:(ko + 1) * P], ident)
                nc.vector.tensor_copy(rqT[ko], tp)
            XTa = big.tile([P, 2, RT], FP16, name="XTa")
            qp = psc.tile([P, 512], FP32, tag="sc")
            for ki in range(2):
                for kin in range(2):
                    nc.tensor.matmul(
                        qp[:, ki * 256:(ki + 1) * 256],
                        wq_sb[:, kin, ki * P:(ki + 1) * P], rqT[kin],
                        start=(kin == 0), stop=(kin == 1))
            nc.vector.tensor_copy(XTa.rearrange("p k r -> p (k r)"), qp)

            # ---------------- retrieve ----------------
            XTb = big.tile([P, 2, RT], FP16, name="XTb")
            X4T = big.tile([P, 2, RT], FP16, name="X4T")
            XTin, XTout = XTa, XTb
            last_tgt = None
            for i in range(4):
                msc = []
                for pr in range(4):
                    sc = psc.tile([P, 512], FP32, tag="sc")
                    for jj in range(2):
                        jt = pr * 2 + jj
                        for ki in range(2):
                            nc.tensor.matmul(
                                sc[:, jj * 256:(jj + 1) * 256],
                                Lf[i][ki][:, jt * P:(jt + 1) * P],
                                XTin[:, ki, :],
                                start=(ki == 0), stop=(ki == 1))
                    m = rot.tile([P, 512], FP16, tag="msc", bufs=4)
                    nc.vector.tensor_mul(m, sc, maskbx)
                    msc.append(m)
                y = pmm.tile([P, 2, RT], FP32, tag="mm")
                for mo in range(2):
                    for ki in range(2):
                        nc.tensor.matmul(
                            y[:, mo, :], w_sb[:, i, ki, mo * P:(mo + 1) * P],
                            XTin[:, ki, :],
                            start=(ki == 0), stop=False)
                    for jt in range(8):
                        nc.tensor.matmul(
                            y[:, mo, :], Rf[i][jt][:, mo * P:(mo + 1) * P],
                            msc[jt // 2][:, (jt % 2) * 256:(jt % 2 + 1) * 256],
                            start=False, stop=(jt == 7))
                if i < 3:
                    # silu via tanh (same act table as exp): X_stored picks up
                    # a 2x per layer, compensated in the next tanh's scale and
                    # cancelled by the scale-invariant postnorm at the end
                    for mo in range(2):
                        tgt = rot.tile([P, RT], FP16, tag="sgr", bufs=2)
                        nc.scalar.activation(tgt, y[:, mo, :], AF.Tanh,
                                             scale=0.5 / (2.0 ** i))
                        last_tgt = tgt
                        nc.vector.scalar_tensor_tensor(
                            out=XTout[:, mo, :], in0=tgt, scalar=1.0,
                            in1=y[:, mo, :], op0=ALU.add, op1=ALU.mult)
                else:
                    nc.vector.tensor_copy(
                        X4T.rearrange("p k r -> p (k r)"), y.rearrange(
                            "p k r -> p (k r)"))
                XTin, XTout = XTout, XTin

            # ---------------- postnorm + output ----------------
            # dummy sqrt depending on the last tanh: pulls the sqrt-set
            # table load off the tail chain but no earlier than retrieve L2
            dmy = rot.tile([P, 1], FP32, tag="dmy", bufs=1)
            nc.scalar.activation(dmy, last_tgt[:, 0:1], AF.Sqrt)
            out_rr = out_d[:].rearrange("(p i) d -> p i d", p=P)
            for rt in range(2):
                tp = ptr.tile([P, 256], FP16, tag="tr")
                for ko in range(2):
                    nc.tensor.transpose(
                        tp[:, ko * P:(ko + 1) * P],
                        X4T[:, ko, rt * P:(rt + 1) * P], ident)
                x4 = rot.tile([P, D], FP32, tag="x4", bufs=2)
                nc.vector.tensor_copy(x4, tp)
                rv = rms_start(x4, "pn")
                rstd = rot.tile([P, 1], FP32, tag="pnrs", bufs=2)
                nc.scalar.activation(rstd, rv, AF.Sqrt)
                o = rot.tile([P, D], FP32, tag="osb", bufs=2)
                nc.vector.tensor_scalar_mul(o, x4, rstd)
                nc.sync.dma_start(out_rr[:, rt, :], o)

    nc.compile()
    return nc


def _host_prep(inputs):
    seq = np.ascontiguousarray(np.asarray(inputs["seq"], dtype=np.float32))
    Wq = np.asarray(inputs["Wq"], dtype=np.float32)
    Wkv = np.asarray(inputs["Wkv"], dtype=np.float32)
    Ws = [np.asarray(inputs[f"W{i}"], dtype=np.float32) for i in range(4)]
    wa = np.asarray(inputs["w_adapt"], dtype=np.float32)
    wm = np.asarray(inputs["w_mom"], dtype=np.float32)
    wd = np.asarray(inputs["w_decay"], dtype=np.float32)

    def kxm(w):  # [K, M] -> [128, (K/128)*M]
        return w.reshape(w.shape[0] // P, P, w.shape[1]).transpose(1, 0, 2) \
            .reshape(P, -1)

    ii = np.arange(N)
    low = (ii[:, None] <= ii[None, :]).astype(np.float32)  # k <= t

    # zp cols: [mom, decay, lr, pad]
    wpack = np.zeros((D, 4), np.float32)
    wpack[:, 0] = wm
    wpack[:, 1] = wd
    wpack[:, 2] = wa
    wpack *= (1.0 / C)

    w32 = np.zeros((P, W32_SZ), np.float32)
    w32[:, WP_O:WP_O + 8] = kxm(wpack)
    w32[:N, UTN_O:UTN_O + N] = low
    w32[:N, NUTN_O:NUTN_O + N] = -low
    mls = np.full((P, N), -1e30, np.float32)
    mls[:N] = np.where(ii[:, None] >= ii[None, :], 0.0, -1e30)
    w32[:, MLS_O:MLS_O + N] = mls
    mut = np.full((P, N), -1e30, np.float32)
    mut[:N] = np.where(ii[:, None] <= ii[None, :], 0.0, -1e30)
    w32[:, MUT_O:MUT_O + N] = mut
    pp = np.arange(P)
    w32[:N, EL_O:EL_O + P] = (ii[:, None] == (pp[None, :] // 2)) \
        .astype(np.float32)

    w16 = np.zeros((P, W16_SZ), np.float16)
    w16[:, WQ_O:WQ_O + 512] = kxm(Wq).astype(np.float16)
    w16[:, WKV_O:WKV_O + 1024] = kxm(Wkv).astype(np.float16)
    w_all = np.stack(Ws).reshape(4, 2, P, D).transpose(2, 0, 1, 3)
    w16[:, W_O:W_O + 2048] = w_all.reshape(P, -1).astype(np.float16)
    wt_all = np.stack([Ws[1].T, Ws[2].T, Ws[3].T]) \
        .reshape(3, 2, P, D).transpose(2, 0, 1, 3)
    w16[:, WT_O:WT_O + 1536] = wt_all.reshape(P, -1).astype(np.float16)
    w16s = np.eye(P, dtype=np.float16)

    rr = np.arange(RT)
    tok = 2 * (rr % P) + rr // P  # retrieve column r -> local query token

    in_maps = []
    for core in range(NCORES):
        b, g = divmod(core, GROUPS)
        j0 = RT * g + (C - 1)
        w32_c = w32.copy()
        gchunk = (RT * g + tok) // C
        iexp = np.zeros((P, RT), np.float32)
        valid = (j0 + tok) < L
        iexp[gchunk[valid], rr[valid]] = 2.0 / D
        w32_c[:, IEXP_O:IEXP_O + RT] = iexp
        qs = np.zeros((RT, D), np.float32)
        src = seq[b, j0:min(j0 + RT, L)]
        qs[:len(src)] = src
        in_maps.append({"w32_d": w32_c, "w16_d": w16, "w16s_d": w16s,
                        "seq_b": seq[b], "seq_q": qs})
    return in_maps


def kernel(**inputs):
    if "nc" not in _CACHE:
        _CACHE["nc"] = _build()
    nc = _CACHE["nc"]
    in_maps = _host_prep(inputs)
    trace = bool(int(os.environ.get("KERNEL_TRACE", "0")))
    if trace:
        try:
            from antenv.axon_hooks import get_axon_ntff_profile_hook  # noqa: F401
        except ImportError:
            trace = _install_ntff_hook()
    res = run_bass_kernel_spmd(
        nc, in_maps, core_ids=list(range(NCORES)), trace=trace)
    LAST_PERF.clear()
    LAST_PERF.update(dict(
        exec_time_ns=res.exec_time_ns,
        mean_exec_time_ns=res.mean_exec_time_ns,
        profile_json=res.profile_json,
        trace=res.instructions_and_trace[1] if res.instructions_and_trace else None,
    ))
    final = np.zeros((B, L, D), np.float32)
    rr = np.arange(RT)
    tok = 2 * (rr % P) + rr // P
    for core in range(NCORES):
        b, g = divmod(core, GROUPS)
        j0 = RT * g + (C - 1)
        n = min(RT, L - j0)
        # out rows are already in token order via the permuted store AP
        final[b, j0:j0 + n] = res.results[core]["out"][:n]
    return final
